# revision 12
# baseline (speedup 1.0000x reference)
"""EMS loss (margin-scaled cross-entropy, mean reduction) on 8 TRN2 NeuronCores.

v4: fp8 streaming, hybrid-layout multi-engine exp.

The f32 HBM floor is 183 us/core; inputs are downcast to fp8 e4m3 on the
host (S = sum exp tolerates ~1e-3 rel error; the final gate is 2e-2), so
the stream is 16.38 MB/core (~45 us at the measured 368 GB/s, single sync
HWDGE queue, >=10KB per-partition descriptors). Under concurrent full-rate
DMA, SBUF port contention limits any single engine (in-situ ACT exp ~0.98
ns/col, DVE ~0.6 ns/col vs 0.25/1.04 resident), so exp is split:

- ACT (36% of columns): normal row-block layout xa[p, rb*CA+c], table exp
  with fused per-row accumulate (acc -> s[128,4], PE-transposed to [1,512]).
- DVE (64%): transposed-blocked xc[p, t*512+r] = x[r, col], Schraudolph
  fast exp y=(x+K)*C1 -> int8 whose bits are the e5m2 exp (round-to-nearest
  conversion verified on HW); PE accumulates ones^T @ chunk into PSUM
  [1,512] across all blocks.

Target logits are gathered exactly from a resident f32 copy (512
elems/core, sensitivity of nll to v is ~4 so fp8 would be too coarse),
PE-transposed into the [1,512] row layout. Tail: S' = S - e^v + e^{4v},
nll = log S' - 4v, mean via free-dim reduce, AllReduce over 8 cores.
"""

import os
import sys

sys.path.insert(0, "/opt/trn_rl_repo")

import numpy as np
import ml_dtypes

import concourse.bacc as bacc
import concourse.bass as bass
import concourse.mybir as mybir
import concourse.tile as tile
from concourse.bass_utils import run_bass_kernel_spmd

N_CORES = 8
B = 4096            # global batch
V = 32000           # vocab
RPC = B // N_CORES  # rows per core = 512
P = 128             # SBUF partitions
RB = RPC // P       # row blocks per core = 4
MARGIN = 4.0

CA = 11520                  # ACT columns (multiple of 128)
NTC = (V - CA) // P         # DVE col-blocks = 160
# ACT chunk widths per row block (bytes/partition = width); last block tapers
ACT_CHUNKS = [[6400, 5120]] * (RB - 1) + [[6400, 2560, 1280, 1280]]
# DVE tile sizes in col-blocks; last tiles taper
KS = [26] * 5 + [16, 9, 5]

# Schraudolph fast-exp: y = (x + K)*C1 -> int8; bits are the e5m2 exp.
# c=0.25 calibrated: mean rel err of sum(exp) on N(0,1) fp8 inputs ~ -2e-3.
SCHRAU_C1 = float(np.float32(2**2 * np.log2(np.e)))
SCHRAU_K = float(np.float32((15 * 2**2 - 0.25) / SCHRAU_C1))

_cache = {}


def _dma_order():
    """Merge ACT-chunk and DVE-tile DMA issue lists by progress fraction so
    both consumers stay fed from the single FIFO queue."""
    acts = []
    for rb, chunks in enumerate(ACT_CHUNKS):
        off = 0
        for w in chunks:
            acts.append(("a", rb, off, w))
            off += w
    dves = []
    t0 = 0
    for K in KS:
        dves.append(("c", t0, K))
        t0 += K
    a_tot = sum(sum(c) for c in ACT_CHUNKS)
    c_tot = sum(KS) * RPC
    order = []
    ai = ci = 0
    a_done = c_done = 0
    while ai < len(acts) or ci < len(dves):
        if ci >= len(dves):
            take_a = True
        elif ai >= len(acts):
            take_a = False
        else:
            take_a = (a_done / a_tot) <= (c_done / c_tot)
        if take_a:
            order.append(acts[ai])
            a_done += acts[ai][3]
            ai += 1
        else:
            order.append(dves[ci])
            c_done += dves[ci][2] * RPC
            ci += 1
    return order


def _build(repeats=1, tail_every_rep=True):
    nc = bacc.Bacc(
        "TRN2",
        target_bir_lowering=False,
        debug=False,
        num_devices=N_CORES,
    )
    f32 = mybir.dt.float32
    i32 = mybir.dt.int32
    fp8 = mybir.dt.float8e4
    e5 = mybir.dt.float8e5
    i8 = mybir.dt.int8

    xa = nc.dram_tensor("xa", [P, RB * CA], fp8, kind="ExternalInput").ap()
    xc = nc.dram_tensor("xc", [P, NTC * RPC], fp8, kind="ExternalInput").ap()
    xf = nc.dram_tensor("xf", [RPC, V], f32, kind="ExternalInput").ap()
    tgt = nc.dram_tensor("targets", [P, RB], i32, kind="ExternalInput").ap()
    out = nc.dram_tensor("out", [1, 1], f32, kind="ExternalOutput").ap()
    cc_in = nc.dram_tensor("cc_in", [1, 1], f32).ap()
    cc_out = nc.dram_tensor("cc_out", [1, 1], f32).ap()

    order = _dma_order()
    n_act_chunks = sum(len(c) for c in ACT_CHUNKS)
    kmax = max(KS)
    wmax = max(max(c) for c in ACT_CHUNKS)

    with tile.TileContext(nc) as tc:
        with (
            tc.tile_pool(name="xpa", bufs=3) as xpa,
            tc.tile_pool(name="xpc", bufs=3) as xpc,
            tc.tile_pool(name="scr", bufs=2) as scr,
            tc.tile_pool(name="small", bufs=1) as small,
            tc.tile_pool(name="ps", bufs=1, space="PSUM") as ps,
        ):
          onesb = small.tile([P, 1], e5)
          nc.vector.memset(onesb[:], 1.0)
          # identity for PE transposes ([128,4] <-> [4,128] reshapes)
          fr = small.tile([P, P], i32)
          nc.gpsimd.iota(fr[:], pattern=[[1, P]], base=0, channel_multiplier=0)
          pc = small.tile([P, 1], i32)
          nc.gpsimd.iota(pc[:], pattern=[[0, 1]], base=0, channel_multiplier=1)
          frf = small.tile([P, P], f32)
          nc.vector.tensor_copy(out=frf[:], in_=fr[:])
          pcf = small.tile([P, 1], f32)
          nc.vector.tensor_copy(out=pcf[:], in_=pc[:])
          ident = small.tile([P, P], f32)
          nc.vector.tensor_scalar(
              out=ident[:], in0=frf[:], scalar1=pcf[:, 0:1], scalar2=None,
              op0=mybir.AluOpType.is_equal)
          for _rep in range(repeats):
           is_last = _rep == repeats - 1
           run_tail = tail_every_rep or is_last
           if run_tail:
            # ---- exact target-logit gather into [128, 4], then -> [1, 512]
            tgt_s = small.tile([P, RB], i32)
            nc.gpsimd.dma_start(out=tgt_s[:], in_=tgt)
            base = small.tile([P, RB], i32)
            nc.gpsimd.iota(base[:], pattern=[[P, RB]], base=0,
                           channel_multiplier=1)
            nc.gpsimd.tensor_scalar(
                out=base[:], in0=base[:], scalar1=V, scalar2=None,
                op0=mybir.AluOpType.mult)
            idx = small.tile([P, RB], i32)
            nc.gpsimd.tensor_tensor(
                out=idx[:], in0=tgt_s[:], in1=base[:], op=mybir.AluOpType.add)
            xf_flat = xf.rearrange("a (b c) -> (a b) c", c=1000)
            v = small.tile([P, RB], f32)
            for r in range(RB):
                nc.gpsimd.indirect_dma_start(
                    out=v[:, r : r + 1],
                    out_offset=None,
                    in_=xf_flat,
                    in_offset=bass.IndirectOffsetOnAxis(
                        ap=idx[:, r : r + 1], axis=1
                    ),
                )
            vt = ps.tile([RB, P], f32)
            nc.tensor.transpose(out=vt[:], in_=v[:], identity=ident[:])
            v4 = small.tile([RB, P], f32)
            nc.vector.tensor_copy(out=v4[:], in_=vt[:])
            v_l = small.tile([1, RPC], f32)
            for r in range(RB):
                nc.gpsimd.dma_start(
                    out=v_l[0:1, r * P : (r + 1) * P], in_=v4[r : r + 1, :])

           # ---- streaming: ACT chunks (accum) + DVE tiles (PE-reduced)
           pt = ps.tile([1, RPC], f32)
           acc = small.tile([P, n_act_chunks], f32)
           et = scr.tile([P, wmax], fp8)
           last_stream_act = None
           ak = 0
           n_mm = 0
           for item in order:
                if item[0] == "a":
                    _, rb, off, w = item
                    at = xpa.tile([P, wmax], fp8, tag="at")
                    nc.sync.dma_start(
                        out=at[:, :w],
                        in_=xa[:, rb * CA + off : rb * CA + off + w])
                    last_stream_act = nc.scalar.activation(
                        out=et[:, :w], in_=at[:, :w],
                        func=mybir.ActivationFunctionType.Exp,
                        accum_out=acc[:, ak : ak + 1])
                    ak += 1
                else:
                    _, t0, K = item
                    ct = xpc.tile([P, kmax * RPC], fp8, tag="ct")
                    nc.sync.dma_start(
                        out=ct[:, : K * RPC],
                        in_=xc[:, t0 * RPC : (t0 + K) * RPC])
                    yt = scr.tile([P, kmax * RPC], i8, tag="yt")
                    nc.vector.tensor_scalar(
                        out=yt[:, : K * RPC],
                        in0=ct[:, : K * RPC],
                        scalar1=SCHRAU_K, scalar2=SCHRAU_C1,
                        op0=mybir.AluOpType.add, op1=mybir.AluOpType.mult)
                    for b in range(K):
                        nc.tensor.matmul(
                            out=pt[:], lhsT=onesb[:],
                            rhs=yt[:, b * RPC : (b + 1) * RPC].bitcast(e5),
                            start=(n_mm == 0), stop=(n_mm == NTC - 1))
                        n_mm += 1

           # ---- tail
           if run_tail:
            s = small.tile([P, RB], f32)
            ak = 0
            for rb, chunks in enumerate(ACT_CHUNKS):
                nc.vector.reduce_sum(
                    out=s[:, rb : rb + 1],
                    in_=acc[:, ak : ak + len(chunks)],
                    axis=mybir.AxisListType.X)
                ak += len(chunks)
            st = ps.tile([RB, P], f32)
            nc.tensor.transpose(out=st[:], in_=s[:], identity=ident[:])
            s4 = small.tile([RB, P], f32)
            nc.vector.tensor_copy(out=s4[:], in_=st[:])
            s_l = small.tile([1, RPC], f32)
            for r in range(RB):
                nc.gpsimd.dma_start(
                    out=s_l[0:1, r * P : (r + 1) * P], in_=s4[r : r + 1, :])

            ev = small.tile([1, RPC], f32)
            ev_inst = nc.scalar.activation(
                out=ev[:], in_=v_l[:], func=mybir.ActivationFunctionType.Exp)
            e4 = small.tile([1, RPC], f32)
            e4_inst = nc.scalar.activation(
                out=e4[:], in_=v_l[:], func=mybir.ActivationFunctionType.Exp,
                scale=MARGIN)
            if last_stream_act is not None:
                from concourse.tile import add_dep_helper

                for inst in (ev_inst, e4_inst):
                    add_dep_helper(
                        inst.ins, last_stream_act.ins, sync=False,
                        reason="tail exps after streaming exps")
            sp = small.tile([1, RPC], f32)
            nc.vector.tensor_tensor(
                out=sp[:], in0=pt[:], in1=s_l[:], op=mybir.AluOpType.add)
            nc.vector.tensor_tensor(
                out=sp[:], in0=sp[:], in1=ev[:], op=mybir.AluOpType.subtract)
            nc.vector.tensor_tensor(
                out=sp[:], in0=sp[:], in1=e4[:], op=mybir.AluOpType.add)
            lg = small.tile([1, RPC], f32)
            nc.scalar.activation(
                out=lg[:], in_=sp[:], func=mybir.ActivationFunctionType.Ln)
            w4 = small.tile([1, RPC], f32)
            nc.vector.tensor_scalar_mul(w4[:], v_l[:], MARGIN)
            nll = small.tile([1, RPC], f32)
            nc.vector.tensor_tensor(
                out=nll[:], in0=lg[:], in1=w4[:], op=mybir.AluOpType.subtract)
            rs = small.tile([1, 1], f32)
            nc.vector.reduce_sum(
                out=rs[:], in_=nll[:], axis=mybir.AxisListType.X)
            res = small.tile([1, 1], f32)
            nc.scalar.mul(out=res[:], in_=rs[:], mul=1.0 / B)

            nc.sync.dma_start(out=cc_in, in_=res[:])
            nc.gpsimd.collective_compute(
                "AllReduce",
                mybir.AluOpType.add,
                replica_groups=[list(range(N_CORES))],
                ins=[cc_in.opt()],
                outs=[cc_out.opt()],
            )
            nc.sync.dma_start(out=out, in_=cc_out)

    # Pre-place one ACT table load of a set containing Exp AND Ln so the
    # auto-pass doesn't switch tables mid-kernel (~2.7us per switch).
    try:
        from concourse.hw_specs import get_activation_tables

        tables = get_activation_tables(nc.m.arch)
        need = {
            mybir.ActivationFunctionType.Exp,
            mybir.ActivationFunctionType.Ln,
            mybir.ActivationFunctionType.Copy,
        }
        set_id = next(
            i for i, funcs in enumerate(tables.values()) if need <= funcs
        )
        inst = mybir.InstLoadActFuncSet(
            name=nc.get_next_instruction_name(),
            act_func_set_id=set_id,
            ins=[],
            outs=[],
        )
        inst.engine = mybir.EngineType.Activation
        nc.register_instruction(inst)
        nc.main_func.blocks[0].instructions.insert(0, inst)
    except (ImportError, StopIteration):
        pass

    nc.compile()
    return nc


def _prep_in_maps(x, t):
    """x [4096, 32000] f32, t [4096] int -> per-core input dicts."""
    in_maps = []
    x8_full = x.astype(ml_dtypes.float8_e4m3)
    for i in range(N_CORES):
        xs = x[i * RPC : (i + 1) * RPC]
        x8 = x8_full[i * RPC : (i + 1) * RPC]
        # ACT share, normal row-block layout: xa[p, rb*CA + c] = x8[rb*128+p, c]
        xa = np.ascontiguousarray(
            x8[:, :CA].reshape(RB, P, CA).transpose(1, 0, 2).reshape(P, RB * CA)
        )
        # DVE share, transposed-blocked: xc[p, t*512 + r] = x8[r, CA + t*128 + p]
        xc = np.ascontiguousarray(
            x8[:, CA:].reshape(RPC, NTC, P).transpose(2, 1, 0).reshape(P, NTC * RPC)
        )
        ts = np.ascontiguousarray(
            t[i * RPC : (i + 1) * RPC].astype(np.int32).reshape(RB, P).T
        )
        in_maps.append(
            {"xa": xa, "xc": xc, "xf": np.ascontiguousarray(xs), "targets": ts}
        )
    return in_maps


def kernel(**inputs):
    x = np.ascontiguousarray(inputs["inputs"], dtype=np.float32)
    t = np.asarray(inputs["targets"])
    assert x.shape == (B, V), x.shape

    if "nc" not in _cache:
        _cache["nc"] = _build()
    nc = _cache["nc"]

    in_maps = _prep_in_maps(x, t)
    results = run_bass_kernel_spmd(
        nc,
        in_maps,
        core_ids=list(range(N_CORES)),
        trace=bool(int(os.environ.get("EMS_TRACE", "0"))),
    )
    _cache["last_results"] = results
    return np.asarray(results.results[0]["out"][0, 0], dtype=np.float32)


# revision 13
# speedup vs baseline: 1.1215x; 1.1215x over previous
"""EMS loss (margin-scaled cross-entropy, mean reduction) on 8 TRN2 NeuronCores.

v4: fp8 streaming, hybrid-layout multi-engine exp.

The f32 HBM floor is 183 us/core; inputs are downcast to fp8 e4m3 on the
host (S = sum exp tolerates ~1e-3 rel error; the final gate is 2e-2), so
the stream is 16.38 MB/core (~45 us at the measured 368 GB/s, single sync
HWDGE queue, >=10KB per-partition descriptors). Under concurrent full-rate
DMA, SBUF port contention limits any single engine (in-situ ACT exp ~0.98
ns/col, DVE ~0.6 ns/col vs 0.25/1.04 resident), so exp is split:

- ACT (36% of columns): normal row-block layout xa[p, rb*CA+c], table exp
  with fused per-row accumulate (acc -> s[128,4], PE-transposed to [1,512]).
- DVE (64%): transposed-blocked xc[p, t*512+r] = x[r, col], Schraudolph
  fast exp y=(x+K)*C1 -> int8 whose bits are the e5m2 exp (round-to-nearest
  conversion verified on HW); PE accumulates ones^T @ chunk into PSUM
  [1,512] across all blocks.

Target logits are gathered exactly from a resident f32 copy (512
elems/core, sensitivity of nll to v is ~4 so fp8 would be too coarse),
PE-transposed into the [1,512] row layout. Tail: S' = S - e^v + e^{4v},
nll = log S' - 4v, mean via free-dim reduce, AllReduce over 8 cores.
"""

import os
import sys

sys.path.insert(0, "/opt/trn_rl_repo")

import numpy as np
import ml_dtypes

import concourse.bacc as bacc
import concourse.bass as bass
import concourse.mybir as mybir
import concourse.tile as tile
from concourse.bass_utils import run_bass_kernel_spmd

N_CORES = 8
B = 4096            # global batch
V = 32000           # vocab
RPC = B // N_CORES  # rows per core = 512
P = 128             # SBUF partitions
RB = RPC // P       # row blocks per core = 4
MARGIN = 4.0

CA = 11520                  # ACT columns (multiple of 128)
NTC = (V - CA) // P         # DVE col-blocks = 160
# ACT chunk widths per row block (bytes/partition = width); last block tapers
ACT_CHUNKS = [[6400, 5120]] * (RB - 1) + [[6400, 2560, 1280, 1280]]
# DVE tile sizes in col-blocks; last tiles taper
KS = [26] * 5 + [16, 9, 5]

# Schraudolph fast-exp: y = (x + K)*C1 -> int8; bits are the e5m2 exp.
# c=0.25 calibrated: mean rel err of sum(exp) on N(0,1) fp8 inputs ~ -2e-3.
SCHRAU_C1 = float(np.float32(2**2 * np.log2(np.e)))
SCHRAU_K = float(np.float32((15 * 2**2 - 0.25) / SCHRAU_C1))

_cache = {}


def _dma_order():
    """Merge ACT-chunk and DVE-tile DMA issue lists by progress fraction so
    both consumers stay fed from the single FIFO queue."""
    acts = []
    for rb, chunks in enumerate(ACT_CHUNKS):
        off = 0
        for w in chunks:
            acts.append(("a", rb, off, w))
            off += w
    dves = []
    t0 = 0
    for K in KS:
        dves.append(("c", t0, K))
        t0 += K
    a_tot = sum(sum(c) for c in ACT_CHUNKS)
    c_tot = sum(KS) * RPC
    order = []
    ai = ci = 0
    a_done = c_done = 0
    while ai < len(acts) or ci < len(dves):
        if ci >= len(dves):
            take_a = True
        elif ai >= len(acts):
            take_a = False
        else:
            take_a = (a_done / a_tot) <= (c_done / c_tot)
        if take_a:
            order.append(acts[ai])
            a_done += acts[ai][3]
            ai += 1
        else:
            order.append(dves[ci])
            c_done += dves[ci][2] * RPC
            ci += 1
    return order


def _build(repeats=1, tail_every_rep=True):
    nc = bacc.Bacc(
        "TRN2",
        target_bir_lowering=False,
        debug=False,
        num_devices=N_CORES,
    )
    f32 = mybir.dt.float32
    i32 = mybir.dt.int32
    fp8 = mybir.dt.float8e4
    e5 = mybir.dt.float8e5
    i8 = mybir.dt.int8

    xa = nc.dram_tensor("xa", [P, RB * CA], fp8, kind="ExternalInput").ap()
    xc = nc.dram_tensor("xc", [P, NTC * RPC], fp8, kind="ExternalInput").ap()
    xf = nc.dram_tensor("xf", [RPC, V], f32, kind="ExternalInput").ap()
    tgt = nc.dram_tensor("targets", [P, RB], i32, kind="ExternalInput").ap()
    out = nc.dram_tensor("out", [1, 1], f32, kind="ExternalOutput").ap()
    cc_in = nc.dram_tensor("cc_in", [1, 1], f32).ap()
    cc_out = nc.dram_tensor("cc_out", [1, 1], f32).ap()

    order = _dma_order()
    n_act_chunks = sum(len(c) for c in ACT_CHUNKS)
    kmax = max(KS)
    wmax = max(max(c) for c in ACT_CHUNKS)

    with tile.TileContext(nc) as tc:
        with (
            tc.tile_pool(name="xpa", bufs=4) as xpa,
            tc.tile_pool(name="xpc", bufs=4) as xpc,
            tc.tile_pool(name="scr", bufs=2) as scr,
            tc.tile_pool(name="small", bufs=1) as small,
            tc.tile_pool(name="ps", bufs=1, space="PSUM") as ps,
        ):
          onesb = small.tile([P, 1], e5)
          nc.vector.memset(onesb[:], 1.0)
          # identity for PE transposes ([128,4] <-> [4,128] reshapes)
          fr = small.tile([P, P], i32)
          nc.gpsimd.iota(fr[:], pattern=[[1, P]], base=0, channel_multiplier=0)
          pc = small.tile([P, 1], i32)
          nc.gpsimd.iota(pc[:], pattern=[[0, 1]], base=0, channel_multiplier=1)
          frf = small.tile([P, P], f32)
          nc.vector.tensor_copy(out=frf[:], in_=fr[:])
          pcf = small.tile([P, 1], f32)
          nc.vector.tensor_copy(out=pcf[:], in_=pc[:])
          ident = small.tile([P, P], f32)
          nc.vector.tensor_scalar(
              out=ident[:], in0=frf[:], scalar1=pcf[:, 0:1], scalar2=None,
              op0=mybir.AluOpType.is_equal)
          for _rep in range(repeats):
           is_last = _rep == repeats - 1
           run_tail = tail_every_rep or is_last
           if run_tail:
            # ---- exact target-logit gather into [128, 4], then -> [1, 512]
            tgt_s = small.tile([P, RB], i32)
            nc.gpsimd.dma_start(out=tgt_s[:], in_=tgt)
            base = small.tile([P, RB], i32)
            nc.gpsimd.iota(base[:], pattern=[[P, RB]], base=0,
                           channel_multiplier=1)
            nc.gpsimd.tensor_scalar(
                out=base[:], in0=base[:], scalar1=V, scalar2=None,
                op0=mybir.AluOpType.mult)
            idx = small.tile([P, RB], i32)
            nc.gpsimd.tensor_tensor(
                out=idx[:], in0=tgt_s[:], in1=base[:], op=mybir.AluOpType.add)
            xf_flat = xf.rearrange("a (b c) -> (a b) c", c=1000)
            v = small.tile([P, RB], f32)
            for r in range(RB):
                nc.gpsimd.indirect_dma_start(
                    out=v[:, r : r + 1],
                    out_offset=None,
                    in_=xf_flat,
                    in_offset=bass.IndirectOffsetOnAxis(
                        ap=idx[:, r : r + 1], axis=1
                    ),
                )
            vt = ps.tile([RB, P], f32)
            nc.tensor.transpose(out=vt[:], in_=v[:], identity=ident[:])
            v4 = small.tile([RB, P], f32)
            nc.vector.tensor_copy(out=v4[:], in_=vt[:])
            v_l = small.tile([1, RPC], f32)
            for r in range(RB):
                nc.gpsimd.dma_start(
                    out=v_l[0:1, r * P : (r + 1) * P], in_=v4[r : r + 1, :])

           # ---- streaming: ACT chunks (accum) + DVE tiles (PE-reduced)
           pt = ps.tile([1, RPC], f32)
           acc = small.tile([P, n_act_chunks], f32)
           et = scr.tile([P, wmax], fp8)
           last_stream_act = None
           ak = 0
           n_mm = 0
           for item in order:
                if item[0] == "a":
                    _, rb, off, w = item
                    at = xpa.tile([P, wmax], fp8, tag="at")
                    nc.sync.dma_start(
                        out=at[:, :w],
                        in_=xa[:, rb * CA + off : rb * CA + off + w])
                    last_stream_act = nc.scalar.activation(
                        out=et[:, :w], in_=at[:, :w],
                        func=mybir.ActivationFunctionType.Exp,
                        accum_out=acc[:, ak : ak + 1])
                    ak += 1
                else:
                    _, t0, K = item
                    ct = xpc.tile([P, kmax * RPC], fp8, tag="ct")
                    nc.sync.dma_start(
                        out=ct[:, : K * RPC],
                        in_=xc[:, t0 * RPC : (t0 + K) * RPC])
                    yt = scr.tile([P, kmax * RPC], i8, tag="yt")
                    nc.vector.tensor_scalar(
                        out=yt[:, : K * RPC],
                        in0=ct[:, : K * RPC],
                        scalar1=SCHRAU_K, scalar2=SCHRAU_C1,
                        op0=mybir.AluOpType.add, op1=mybir.AluOpType.mult)
                    for b in range(K):
                        nc.tensor.matmul(
                            out=pt[:], lhsT=onesb[:],
                            rhs=yt[:, b * RPC : (b + 1) * RPC].bitcast(e5),
                            start=(n_mm == 0), stop=(n_mm == NTC - 1))
                        n_mm += 1

           # ---- tail
           if run_tail:
            s = small.tile([P, RB], f32)
            ak = 0
            for rb, chunks in enumerate(ACT_CHUNKS):
                nc.vector.reduce_sum(
                    out=s[:, rb : rb + 1],
                    in_=acc[:, ak : ak + len(chunks)],
                    axis=mybir.AxisListType.X)
                ak += len(chunks)
            st = ps.tile([RB, P], f32)
            nc.tensor.transpose(out=st[:], in_=s[:], identity=ident[:])
            s4 = small.tile([RB, P], f32)
            nc.vector.tensor_copy(out=s4[:], in_=st[:])
            s_l = small.tile([1, RPC], f32)
            for r in range(RB):
                nc.gpsimd.dma_start(
                    out=s_l[0:1, r * P : (r + 1) * P], in_=s4[r : r + 1, :])

            ev = small.tile([1, RPC], f32)
            ev_inst = nc.scalar.activation(
                out=ev[:], in_=v_l[:], func=mybir.ActivationFunctionType.Exp)
            e4 = small.tile([1, RPC], f32)
            e4_inst = nc.scalar.activation(
                out=e4[:], in_=v_l[:], func=mybir.ActivationFunctionType.Exp,
                scale=MARGIN)
            if last_stream_act is not None:
                from concourse.tile import add_dep_helper

                for inst in (ev_inst, e4_inst):
                    add_dep_helper(
                        inst.ins, last_stream_act.ins, sync=False,
                        reason="tail exps after streaming exps")
            sp = small.tile([1, RPC], f32)
            nc.vector.tensor_tensor(
                out=sp[:], in0=pt[:], in1=s_l[:], op=mybir.AluOpType.add)
            nc.vector.tensor_tensor(
                out=sp[:], in0=sp[:], in1=ev[:], op=mybir.AluOpType.subtract)
            nc.vector.tensor_tensor(
                out=sp[:], in0=sp[:], in1=e4[:], op=mybir.AluOpType.add)
            lg = small.tile([1, RPC], f32)
            nc.scalar.activation(
                out=lg[:], in_=sp[:], func=mybir.ActivationFunctionType.Ln)
            w4 = small.tile([1, RPC], f32)
            nc.vector.tensor_scalar_mul(w4[:], v_l[:], MARGIN)
            nll = small.tile([1, RPC], f32)
            nc.vector.tensor_tensor(
                out=nll[:], in0=lg[:], in1=w4[:], op=mybir.AluOpType.subtract)
            rs = small.tile([1, 1], f32)
            nc.vector.reduce_sum(
                out=rs[:], in_=nll[:], axis=mybir.AxisListType.X)
            res = small.tile([1, 1], f32)
            nc.scalar.mul(out=res[:], in_=rs[:], mul=1.0 / B)

            nc.sync.dma_start(out=cc_in, in_=res[:])
            nc.gpsimd.collective_compute(
                "AllReduce",
                mybir.AluOpType.add,
                replica_groups=[list(range(N_CORES))],
                ins=[cc_in.opt()],
                outs=[cc_out.opt()],
            )
            nc.sync.dma_start(out=out, in_=cc_out)

    # Pre-place one ACT table load of a set containing Exp AND Ln so the
    # auto-pass doesn't switch tables mid-kernel (~2.7us per switch).
    try:
        from concourse.hw_specs import get_activation_tables

        tables = get_activation_tables(nc.m.arch)
        need = {
            mybir.ActivationFunctionType.Exp,
            mybir.ActivationFunctionType.Ln,
            mybir.ActivationFunctionType.Copy,
        }
        set_id = next(
            i for i, funcs in enumerate(tables.values()) if need <= funcs
        )
        inst = mybir.InstLoadActFuncSet(
            name=nc.get_next_instruction_name(),
            act_func_set_id=set_id,
            ins=[],
            outs=[],
        )
        inst.engine = mybir.EngineType.Activation
        nc.register_instruction(inst)
        nc.main_func.blocks[0].instructions.insert(0, inst)
    except (ImportError, StopIteration):
        pass

    nc.compile()
    return nc


def _prep_in_maps(x, t):
    """x [4096, 32000] f32, t [4096] int -> per-core input dicts."""
    in_maps = []
    x8_full = x.astype(ml_dtypes.float8_e4m3)
    for i in range(N_CORES):
        xs = x[i * RPC : (i + 1) * RPC]
        x8 = x8_full[i * RPC : (i + 1) * RPC]
        # ACT share, normal row-block layout: xa[p, rb*CA + c] = x8[rb*128+p, c]
        xa = np.ascontiguousarray(
            x8[:, :CA].reshape(RB, P, CA).transpose(1, 0, 2).reshape(P, RB * CA)
        )
        # DVE share, transposed-blocked: xc[p, t*512 + r] = x8[r, CA + t*128 + p]
        xc = np.ascontiguousarray(
            x8[:, CA:].reshape(RPC, NTC, P).transpose(2, 1, 0).reshape(P, NTC * RPC)
        )
        ts = np.ascontiguousarray(
            t[i * RPC : (i + 1) * RPC].astype(np.int32).reshape(RB, P).T
        )
        in_maps.append(
            {"xa": xa, "xc": xc, "xf": np.ascontiguousarray(xs), "targets": ts}
        )
    return in_maps


def kernel(**inputs):
    x = np.ascontiguousarray(inputs["inputs"], dtype=np.float32)
    t = np.asarray(inputs["targets"])
    assert x.shape == (B, V), x.shape

    if "nc" not in _cache:
        _cache["nc"] = _build()
    nc = _cache["nc"]

    in_maps = _prep_in_maps(x, t)
    results = run_bass_kernel_spmd(
        nc,
        in_maps,
        core_ids=list(range(N_CORES)),
        trace=bool(int(os.environ.get("EMS_TRACE", "0"))),
    )
    _cache["last_results"] = results
    return np.asarray(results.results[0]["out"][0, 0], dtype=np.float32)
